# revision 1
# baseline (speedup 1.0000x reference)
"""Trainium2 Bass kernel for nn_AttentionSpace_87729001988510.

Batched channel-attention: 3 depthwise convs (K=7) over L, score = QK^T over
L (contracting L), softmax over channels, out = attn @ V.

Sharding: data-parallel over batch B=8 across the 8 NeuronCores (one batch
element per core). Everything below describes the per-core program.

Per-core pipeline (C=1024 channels, L=4096, P=128 partitions):
  Phase A (conv): for each 128-channel chunk, stream x from DRAM, cast to
    fp16 (plus a one-element-shifted copy so every DVE tap is 4B-aligned).
    q is computed on the vector engine via fused (x*w + acc) taps and spilled
    to DRAM in natural layout. k and v are computed on the tensor engine via
    7 diag(w_tap)-stationary matmuls per 512-wide l-block accumulated in
    PSUM; k is corner-turned into a resident transposed kT[l, d] via DMA-xbar
    transposes, v stays resident in natural layout.
  Phase B (score+softmax): S[c, d] accumulated over 32 l-chunks with qT tiles
    streamed from DRAM via transpose-reads as the stationary operand and the
    resident kT as the moving operand. Softmax is along the free axis d:
    row-max on the vector engine, exp((S - max)/sqrt(C)) with fused row-sum
    (accum_out) on the scalar engine, reciprocal, then A = E * recip. A is
    corner-turned into resident A^T via 8 DMA-xbar transposes per chunk.
  Phase C (out): out[c, l] = sum_d A^T[d, c] * v[d, l] with A^T chunks
    stationary; plain copy out of PSUM and DMA to DRAM.
"""

import numpy as np

import concourse.bass as bass
import concourse.tile as tile
from concourse import bacc, mybir
from concourse.bass_utils import run_bass_kernel_spmd

B = 8
C = 1024
L = 4096
K = 7
PAD = 3
P = 128

NCC = C // P  # channel chunks (8)
NLC = L // P  # l chunks for transposes (32)
LB = 512  # l block for conv / out matmuls
NLB = L // LB  # 8
LH = L // 2

INV_SQRT_C = 1.0 / np.sqrt(np.float32(C))

f32 = mybir.dt.float32
f16 = mybir.dt.float16
AF = mybir.ActivationFunctionType
ALU = mybir.AluOpType

# v-conv chunks >= V_DVE_START run on the vector engine overlapped with phase B
V_DVE_START = 5
QW = 1024  # quarter width for the overlapped DVE v-conv


def _build():
    nc = bacc.Bacc("TRN2", target_bir_lowering=False, debug=False)

    x_in = nc.dram_tensor("x", [C, L], f32, kind="ExternalInput").ap()
    wq_in = nc.dram_tensor("wq", [C, K], f32, kind="ExternalInput").ap()
    wv_in = nc.dram_tensor("wv", [C, K], f32, kind="ExternalInput").ap()
    dq_in = nc.dram_tensor("dq", [C, K * P], f16, kind="ExternalInput").ap()
    dk_in = nc.dram_tensor("dk", [C, K * P], f16, kind="ExternalInput").ap()
    dv_in = nc.dram_tensor("dv", [C, K * P], f16, kind="ExternalInput").ap()
    out_dram = nc.dram_tensor("out", [C, L], f32, kind="ExternalOutput").ap()
    qnat_dram = nc.dram_tensor("q_nat_spill", [C, L], f16).ap()
    knat_dram = nc.dram_tensor("k_nat_spill", [C, L], f16).ap()
    a_dram = nc.dram_tensor("a_spill", [C, C], f16).ap()

    Q_PE_START = 6  # q chunks >= this run on PE (diag matmuls)

    with tile.TileContext(nc) as tc:
        with (
            tc.tile_pool(name="big", bufs=1) as big,
            tc.tile_pool(name="pb_qt", bufs=1) as pb_qt,
        ):
            # resident: kT3[l_lo, lc, d]; v d-chunk di at cols [di*L:(di+1)*L]
            kT3 = big.tile([P, NLC, C], f16)
            vres = big.tile([P, NCC * L], f16)

            # warm up the scalar-engine exp table before it is needed
            warm = big.tile([P, 1], f32)
            nc.vector.memset(warm[:], 0.0)
            nc.scalar.activation(warm[:], warm[:], AF.Exp)

            # ---------------- Phase A: convs ----------------
            with (
                tc.tile_pool(name="pa_x", bufs=2) as pa_x,
                tc.tile_pool(name="pa_misc", bufs=2) as pa_misc,
                tc.tile_pool(name="pa_d", bufs=1) as pa_d,
                tc.tile_pool(name="pa_q", bufs=2) as pa_q,
                tc.tile_pool(name="pa_tmp", bufs=1) as pa_tmp,
                tc.tile_pool(name="pa_stage", bufs=3) as pa_stage,
                tc.tile_pool(name="pa_ps", bufs=4, space="PSUM") as pa_ps,
            ):
                def pe_conv(dmat, xp, dst_dram, dst_sbuf, ci):
                    """7-tap diag conv on PE; per 512 block, psum -> fp16."""
                    for lb in range(NLB):
                        ps = pa_ps.tile([P, LB], f32, tag="cps")
                        for j in range(K):
                            nc.tensor.matmul(
                                ps[:],
                                dmat[:, j * P : (j + 1) * P],
                                xp[:, lb * LB + j : lb * LB + j + LB],
                                start=(j == 0),
                                stop=(j == K - 1),
                            )
                        if dst_sbuf is not None:
                            nc.scalar.copy(
                                dst_sbuf[:, ci * L + lb * LB : ci * L + (lb + 1) * LB],
                                ps[:],
                            )
                        else:
                            sb = pa_stage.tile([P, LB], f16, tag="kb")
                            nc.scalar.copy(sb[:], ps[:])
                            nc.scalar.dma_start(
                                dst_dram[
                                    ci * P : (ci + 1) * P, lb * LB : (lb + 1) * LB
                                ],
                                sb[:],
                            )

                for ci in range(NCC):
                    # xp[:, t] = x[t - 3] ; xpo[:, t] = x[t - 2]
                    xp = pa_misc.tile([P, L + 2 * PAD], f16, tag="xp")
                    nc.vector.memset(xp[:, 0:PAD], 0.0)
                    nc.vector.memset(xp[:, L + PAD :], 0.0)
                    xpo = pa_misc.tile([P, L + 2 * PAD], f16, tag="xpo")
                    nc.vector.memset(xpo[:, 0 : PAD - 1], 0.0)
                    nc.vector.memset(xpo[:, L + PAD - 1 :], 0.0)
                    for t in range(4):
                        W4 = L // 4
                        xh = pa_x.tile([P, W4], f32, tag="xh")
                        nc.sync.dma_start(
                            xh[:], x_in[ci * P : (ci + 1) * P, t * W4 : (t + 1) * W4]
                        )
                        nc.scalar.copy(
                            xp[:, PAD + t * W4 : PAD + (t + 1) * W4], xh[:]
                        )
                        nc.scalar.copy(
                            xpo[:, PAD - 1 + t * W4 : PAD - 1 + (t + 1) * W4], xh[:]
                        )

                    dmats = {}
                    dspecs = [("dk", dk_in), ("dv", dv_in)]
                    if ci >= Q_PE_START:
                        dspecs.append(("dq", dq_in))
                    for name, dsrc in dspecs:
                        dm = pa_d.tile([P, K * P], f16, tag=name)
                        nc.sync.dma_start(dm[:], dsrc[ci * P : (ci + 1) * P, :])
                        dmats[name] = dm

                    # ---- q ----
                    if ci >= Q_PE_START:
                        pe_conv(dmats["dq"], xp, qnat_dram, None, ci)
                    else:
                        wq = pa_misc.tile([P, K], f32, tag="wq")
                        nc.sync.dma_start(wq[:], wq_in[ci * P : (ci + 1) * P, :])
                        for h in range(2):
                            o = h * LH
                            qa = pa_q.tile([P, LH], f16, tag="qa")
                            qb = pa_q.tile([P, LH], f16, tag="qb")
                            nc.vector.tensor_scalar_mul(
                                qa[:], xp[:, o : o + LH], wq[:, 0:1]
                            )
                            cur, oth = qa, qb
                            for j in range(1, K):
                                srcap = (
                                    xp[:, o + j : o + j + LH]
                                    if j % 2 == 0
                                    else xpo[:, o + j - 1 : o + j - 1 + LH]
                                )
                                tmp = pa_tmp.tile([P, LH], f16, tag="qtmp")
                                nc.vector.tensor_scalar_mul(
                                    tmp[:], srcap, wq[:, j : j + 1]
                                )
                                nc.vector.tensor_add(oth[:], tmp[:], cur[:])
                                cur, oth = oth, cur
                            nc.sync.dma_start(
                                qnat_dram[ci * P : (ci + 1) * P, o : o + LH], cur[:]
                            )

                    # ---- k (spill), v (resident for chunks < V_DVE_START) ----
                    pe_conv(dmats["dk"], xp, knat_dram, None, ci)
                    if ci < V_DVE_START:
                        pe_conv(dmats["dv"], xp, None, vres, ci)
                    # corner-turn this chunk's k into resident kT3[l, lc, d]
                    nc.sync.dma_start_transpose(
                        kT3[:, :, ci * P : (ci + 1) * P],
                        knat_dram[ci * P : (ci + 1) * P, :],
                    )

            # ---------------- Phase B: score + softmax + A^T ----------------
            with tc.tile_pool(name="bigET", bufs=1) as bigET:
                # resident A^T: ET3[d_lo, dj, c]
                ET3 = bigET.tile([P, NCC, C], f16)

                with (
                    tc.tile_pool(name="pa2", bufs=2) as pa2,
                    tc.tile_pool(name="pb_stage", bufs=2) as pb_stage,
                    tc.tile_pool(name="pb_small", bufs=2) as pb_small,
                    tc.tile_pool(name="pb_ps", bufs=3, space="PSUM") as pb_ps,
                ):
                    for ci in range(NCC):
                        # one corner-turn read: qt3[l_lo, lc, c] for this chunk
                        qt3 = pb_qt.tile([P, NLC, P], f16, tag="qt")
                        nc.sync.dma_start_transpose(
                            qt3[:], qnat_dram[ci * P : (ci + 1) * P, :]
                        )
                        sps = pb_ps.tile([P, C], f32, tag="sps")
                        for lc in range(NLC):
                            for hb in range(2):
                                nc.tensor.matmul(
                                    sps[:, hb * 512 : (hb + 1) * 512],
                                    qt3[:, lc, :],
                                    kT3[:, lc, hb * 512 : (hb + 1) * 512],
                                    start=(lc == 0),
                                    stop=(lc == NLC - 1),
                                )
                        m = pb_small.tile([P, 1], f32, tag="m")
                        nc.vector.tensor_reduce(
                            m[:], sps[:], mybir.AxisListType.X, ALU.max
                        )
                        mneg = pb_small.tile([P, 1], f32, tag="mneg")
                        nc.vector.tensor_scalar_mul(
                            mneg[:], m[:], -float(INV_SQRT_C)
                        )
                        rs = pb_small.tile([P, 1], f32, tag="rs")
                        Es = pb_stage.tile([P, C], f16, tag="Es")
                        nc.scalar.activation(
                            Es[:],
                            sps[:],
                            AF.Exp,
                            scale=float(INV_SQRT_C),
                            bias=mneg[:],
                            accum_out=rs[:],
                        )
                        rcp = pb_small.tile([P, 1], f32, tag="rcp")
                        nc.vector.reciprocal(rcp[:], rs[:])
                        As = pb_stage.tile([P, C], f16, tag="As")
                        nc.scalar.activation(
                            As[:], Es[:], AF.Identity, scale=rcp[:]
                        )
                        nc.scalar.dma_start(
                            a_dram[ci * P : (ci + 1) * P, :], As[:]
                        )
                        # incremental corner-turn of A into resident A^T
                        nc.scalar.dma_start_transpose(
                            ET3[:, :, ci * P : (ci + 1) * P],
                            a_dram[ci * P : (ci + 1) * P, :],
                        )

                    # ---- Phase A2: v-conv chunks V_DVE_START.. on DVE ----
                    # overlapped with phase B; pa2 opened alongside B pools
                    for ci in range(V_DVE_START, NCC):
                        wv = pa2.tile([P, K], f32, tag="wv")
                        nc.sync.dma_start(wv[:], wv_in[ci * P : (ci + 1) * P, :])
                        for qi in range(L // QW):
                            lo = qi * QW
                            # xq col t <-> x[lo - 3 + t], t in [0, QW+7)
                            src_lo = max(lo - 3, 0)
                            src_hi = min(lo + QW + 4, L)
                            xq = pa2.tile([P, QW + 7], f32, tag="xq")
                            if lo == 0:
                                nc.vector.memset(xq[:, 0:3], 0.0)
                            if lo + QW == L:
                                nc.vector.memset(xq[:, QW + 3 :], 0.0)
                            nc.gpsimd.dma_start(
                                xq[:, src_lo - (lo - 3) : src_hi - (lo - 3)],
                                x_in[ci * P : (ci + 1) * P, src_lo:src_hi],
                            )
                            # xpq[:, t] = x[lo+t-3]; xpoq[:, t] = x[lo+t-2]
                            xpq = pa2.tile([P, QW + 6], f16, tag="xpq")
                            nc.scalar.copy(xpq[:], xq[:, 0 : QW + 6])
                            xpoq = pa2.tile([P, QW + 6], f16, tag="xpoq")
                            nc.scalar.copy(xpoq[:], xq[:, 1 : QW + 7])
                            va = pa2.tile([P, QW], f16, tag="va")
                            vb = pa2.tile([P, QW], f16, tag="vb")
                            nc.vector.tensor_scalar_mul(
                                va[:], xpq[:, 0:QW], wv[:, 0:1]
                            )
                            cur, oth = va, vb
                            for j in range(1, K):
                                srcap = (
                                    xpq[:, j : j + QW]
                                    if j % 2 == 0
                                    else xpoq[:, j - 1 : j - 1 + QW]
                                )
                                tmp = pa2.tile([P, QW], f16, tag="vtmp")
                                nc.vector.tensor_scalar_mul(
                                    tmp[:], srcap, wv[:, j : j + 1]
                                )
                                dst = (
                                    vres[:, ci * L + lo : ci * L + lo + QW]
                                    if j == K - 1
                                    else oth[:]
                                )
                                nc.vector.tensor_add(dst, tmp[:], cur[:])
                                cur, oth = oth, cur

                # ---------------- Phase C: out ----------------
                with (
                    tc.tile_pool(name="pc_ob", bufs=4) as pc_ob,
                    tc.tile_pool(name="pc_ps", bufs=6, space="PSUM") as pc_ps,
                ):
                    for lb in range(NLB):
                        for ci in range(NCC):
                            ops = pc_ps.tile([P, LB], f32, tag="ops")
                            for dj in range(NCC):
                                nc.tensor.matmul(
                                    ops[:],
                                    ET3[:, dj, ci * P : (ci + 1) * P],
                                    vres[:, dj * L + lb * LB : dj * L + (lb + 1) * LB],
                                    start=(dj == 0),
                                    stop=(dj == NCC - 1),
                                )
                            ob = pc_ob.tile([P, LB], f32, tag="ob")
                            nc.scalar.copy(ob[:], ops[:])
                            nc.sync.dma_start(
                                out_dram[
                                    ci * P : (ci + 1) * P, lb * LB : (lb + 1) * LB
                                ],
                                ob[:],
                            )

    nc.compile()
    return nc


_nc_cache = None


def _get_nc():
    global _nc_cache
    if _nc_cache is None:
        _nc_cache = _build()
    return _nc_cache


def _diag_blocks(w: np.ndarray) -> np.ndarray:
    """w: [C, 1, K] fp32 -> [C, K*P] fp16 where row r, block j has
    diag entry at column j*P + (r % P) equal to w[r, 0, j]."""
    d = np.zeros((C, K * P), np.float16)
    r = np.arange(C)
    for j in range(K):
        d[r, j * P + (r % P)] = w[r, 0, j].astype(np.float16)
    return d


def _in_maps(x, q_w, k_w, v_w):
    x = np.ascontiguousarray(np.asarray(x, dtype=np.float32))
    wq = np.ascontiguousarray(np.asarray(q_w, dtype=np.float32)[:, 0, :])
    wv = np.ascontiguousarray(np.asarray(v_w, dtype=np.float32)[:, 0, :])
    dq = _diag_blocks(np.asarray(q_w))
    dk = _diag_blocks(np.asarray(k_w))
    dv = _diag_blocks(np.asarray(v_w))
    return [
        {"x": np.ascontiguousarray(x[b]), "wq": wq, "wv": wv, "dq": dq, "dk": dk, "dv": dv}
        for b in range(B)
    ]


def kernel(x, q_w, k_w, v_w):
    nc = _get_nc()
    res = run_bass_kernel_spmd(nc, _in_maps(x, q_w, k_w, v_w), list(range(B)))
    out = np.stack([res.results[b]["out"] for b in range(B)]).astype(np.float32)
    return out

